# revision 20
# baseline (speedup 1.0000x reference)
"""GraphConv (dgl norm='both') distributed Bass kernel for 8 trn2 NeuronCores.

out = relu( D_in^{-1/2} A D_out^{-1/2} (h W) + b )

Sharding: nodes range-partitioned across 8 cores (12500 each, padded to
12544 = 98*128). Per core:
  phase 1: out-degrees of own nodes via selection-matmuls over the
           src-bucketed edge list (32-node cells),
  phase 2: x = (h_shard * norm_src) @ W, ones in col 64, written into a
           256B-pitch row layout [12544, 128] bf16 (cols 65..127 pad),
  phase 3: AllGather x -> x_full [100352, 128] bf16 in shared DRAM,
  phase 4: dst-bucketed edges at 128-node (group) granularity: batched
           SWDGE dma_gather instructions fetch the 130B payload rows of
           x_full (window-relative int16 indices; 4 windows of 25088 rows
           so indices fit int16), then one-hot selection matmuls
           accumulate [128, 65] tiles in PSUM (ones column = in-degree),
  phase 5: per group: norm = rsqrt(max(deg,1)), out = relu(agg * norm).

The dma_gather ucode (InstDMAGatherAnt) is constructed directly because
the bass wrapper requires 256B-multiple payloads; the ucode itself only
requires the table PITCH to be a 256B multiple (stride_bytes_256) and
happily moves 130B per row.  Index list layout (measured on HW, the
interpreter's claim is wrong): index i of a queue-q gather lives at
SBUF partition 32*q+16 + i%16, int16 column i//16.  Gathered row i
lands at partition i%128, block i//128 of the output tile.  Gather
instructions are one per (dst-group, window) bucket with num_idxs equal
to the max true edge count over cores (positions past num_idxs in the
last 128-block are skipped by the ucode), spread round-robin over the 4
SWDGE queues, whose descriptor generation runs on distinct Q7 core
pairs (~3.6x measured speedup over one queue).

Edges are bucketed host-side by (dst owner, dst group, src window) with a
shared count profile (max across cores) so all 8 cores run one identical
SPMD program; pad slots gather window row 0 and compare value -1
(all-zero selection column => no contribution).
"""

import sys

if "/opt/trn_rl_repo" not in sys.path:
    sys.path.insert(0, "/opt/trn_rl_repo")

import numpy as np
from contextlib import ExitStack

import concourse.bass as bass
import concourse.bacc as bacc
import concourse.mybir as mybir
import concourse.tile as tile
from concourse import bass_utils

P = 128
NCORES = 8
N_NODES = 100000
IN_DIM = 256
OUT_DIM = 64
D = OUT_DIM + 1          # gathered payload cols: features + ones
PITCH = 128              # x_full row pitch in bf16 elems (256B)
NLOC = N_NODES // NCORES  # 12500
GROUPS = 98               # 128-node groups per core
NPAD = GROUPS * P         # 12544
NFULL = NCORES * NPAD     # 100352
NWIN = 4                  # index windows (int16 range)
WIN = NFULL // NWIN       # 25088 rows per window
CELL = 32                 # phase-1 cell width
NCELLS = NPAD // CELL     # 392
SB = 16                   # chunks per batched S-build (main pass)
SBPRE = 32                # chunks per batched S-build (pre pass)
GRANGE = 14               # groups per gather round (98 = 7*14)
NROUNDS = GROUPS // GRANGE
NQUEUES = 4               # SWDGE queues, round-robin over buckets

F32 = mybir.dt.float32
BF16 = mybir.dt.bfloat16
I32 = mybir.dt.int32
I16 = mybir.dt.int16


def raw_dma_gather(nc, out_ap, in_ap, idxs_ap, num_idxs, elem_size,
                   stride_bytes, queue_num=0):
    """dma_gather with a payload that is not a 256B multiple (ucode only
    needs the row pitch to be); bypasses the bass wrapper's assert."""
    gp = nc.gpsimd
    _in_ap = gp.lower_ap_dma(in_ap, for_custom_bir_dma=True)
    _idxs_ap = gp.lower_ap(idxs_ap)
    _out_ap = gp.lower_ap(out_ap)
    return gp.add_instruction(
        mybir.InstDMAGatherAnt(
            name=gp.bass.get_next_instruction_name(),
            ins=[*_in_ap, _idxs_ap, gp.lower_val_access(gp.to_reg(num_idxs))],
            outs=[_out_ap],
            transpose=False,
            num_idxs=num_idxs,
            elem_size=elem_size,
            stride_bytes_256=stride_bytes // 256,
            gen_mode=0,
            single_packet=True,
            queue_num=queue_num,
            sbuf_tokens_per_rank=0,
            sbuf_free_dim_per_rank=0,
            sbuf_free_dim_pad_per_rank=0,
            sbuf_byte_offset=0,
        )
    )


def make_layout(nch, cmax):
    """Deterministic chunk/bucket layout shared by host marshalling and
    program build.  nch/cmax: [GROUPS, NWIN] chunk counts / max true edge
    counts per (dst group, src window) bucket.

    Returns dict with:
      co[g][q]   : consumption-order position of chunk (g,q,0) (g-major)
      tc_main    : total chunks (S columns), padded to multiple of SB
      gpos[q][g] : gather-order block position of bucket (g,q) in window q
      rblk[r][q] : (start_block, n_blocks) of round r in window q's stream
      tilemax    : max blocks per (round, window) msgs tile
      qn[(g,q)]  : (queue, idx col start) per nonempty bucket
      icols      : idx tile columns (max over queue bands)
    """
    co = np.zeros((GROUPS, NWIN), np.int64)
    t = 0
    for g in range(GROUPS):
        for q in range(NWIN):
            co[g, q] = t
            t += int(nch[g][q])
    tc_true = t
    tc_main = ((t + SB - 1) // SB) * SB

    gpos = np.zeros((NWIN, GROUPS), np.int64)
    rblk = []
    wtot = [0] * NWIN
    for r in range(NROUNDS):
        row = []
        for q in range(NWIN):
            start = wtot[q]
            for g in range(r * GRANGE, (r + 1) * GRANGE):
                gpos[q, g] = wtot[q]
                wtot[q] += int(nch[g][q])
            row.append((start, wtot[q] - start))
        rblk.append(row)
    tilemax = max(n for row in rblk for (_, n) in row)

    qn = {}
    qcol = [0] * NQUEUES
    bctr = 0
    for r in range(NROUNDS):
        for g in range(r * GRANGE, (r + 1) * GRANGE):
            for q in range(NWIN):
                if nch[g][q] == 0:
                    continue
                qq = bctr % NQUEUES
                bctr += 1
                qn[(g, q)] = (qq, qcol[qq])
                qcol[qq] += (int(cmax[g][q]) + 15) // 16
    icols = max(qcol)
    return dict(co=co, tc_true=tc_true, tc_main=tc_main, gpos=gpos,
                rblk=rblk, tilemax=tilemax, qn=qn, icols=icols)


def prepare_inputs(h, src, dst, W, b):
    """Host-side sharding / marshalling (layout only, no graph math)."""
    import ml_dtypes

    src = np.asarray(src).astype(np.int64)
    dst = np.asarray(dst).astype(np.int64)
    h = np.asarray(h, dtype=np.float32)
    W = np.asarray(W, dtype=np.float32)
    b = np.asarray(b, dtype=np.float32)

    owner_s = src // NLOC
    s_loc = (src - owner_s * NLOC).astype(np.int64)
    grow = (owner_s * NPAD + s_loc).astype(np.int64)  # row in padded x_full
    owner_d = dst // NLOC
    d_loc = (dst - owner_d * NLOC).astype(np.int64)

    win = grow // WIN                      # src window 0..3
    wrel = (grow - win * WIN).astype(np.int64)  # window-relative row < 25088

    # ---- main pass: bucket by (dst owner, dst 128-group, src window) ----
    g_of = d_loc // P
    l_of = d_loc - g_of * P                # local idx within group [0,128)

    counts = np.zeros((NCORES, GROUPS, NWIN), np.int64)
    buckets = []
    for k in range(NCORES):
        m = owner_d == k
        gg, qq, ll, ww = g_of[m], win[m], l_of[m], wrel[m]
        key = gg * NWIN + qq
        order = np.argsort(key, kind="stable")
        ll, ww = ll[order], ww[order]
        cnt = np.bincount(key, minlength=GROUPS * NWIN).reshape(GROUPS, NWIN)
        counts[k] = cnt
        starts = np.concatenate([[0], np.cumsum(cnt.ravel())[:-1]]).reshape(
            GROUPS, NWIN
        )
        buckets.append((ll, ww, starts, cnt))
    cmax = counts.max(axis=0)                       # shared profile
    nch = np.ceil(cmax / P).astype(np.int64)
    lay = make_layout(nch, cmax)
    tc_main = lay["tc_main"]
    co, qn, icols = lay["co"], lay["qn"], lay["icols"]

    main_cmp = np.full((NCORES, tc_main * P), -1.0, np.float32)
    idx_arr = np.zeros((NCORES, P, icols), np.int16)
    for k in range(NCORES):
        ll, ww, starts, cnt = buckets[k]
        for g in range(GROUPS):
            for q in range(NWIN):
                n = cnt[g, q]
                if nch[g][q] == 0:
                    continue
                c0 = starts[g, q]
                p0 = co[g, q] * P
                main_cmp[k, p0 : p0 + n] = ll[c0 : c0 + n]
                qq, col0 = qn[(g, q)]
                idxs = np.zeros(int(cmax[g][q]), np.int16)
                idxs[:n] = ww[c0 : c0 + n]
                ii = np.arange(len(idxs))
                idx_arr[k, 32 * qq + 16 + ii % 16, col0 + ii // 16] = idxs

    # ---- degree pre-pass: bucket src-locals by (src owner, src 32-cell) ----
    p_counts = np.zeros((NCORES, NCELLS), np.int64)
    p_data = []
    for k in range(NCORES):
        m = owner_s == k
        sl = s_loc[m]
        cells = (sl // CELL).astype(np.int64)
        cmpv = (sl - cells * CELL).astype(np.float32)
        order = np.argsort(cells, kind="stable")
        p_counts[k] = np.bincount(cells, minlength=NCELLS)
        p_data.append((cells[order], cmpv[order]))
    mch = np.ceil(p_counts / P).max(axis=0).astype(np.int64)
    p_off = np.concatenate([[0], np.cumsum(mch)]) * P
    tc_pre = int(p_off[-1]) // P

    pre_cmp = np.full((NCORES, tc_pre * P), -1.0, np.float32)
    for k in range(NCORES):
        cells_s, cmp_s = p_data[k]
        starts = np.concatenate([[0], np.cumsum(p_counts[k])[:-1]])
        rank = np.arange(cells_s.shape[0]) - starts[cells_s]
        pos = p_off[:-1][cells_s] + rank
        pre_cmp[k][pos] = cmp_s

    # ---- per-core tensors ----
    hT = np.zeros((NCORES, IN_DIM, NPAD), ml_dtypes.bfloat16)
    for k in range(NCORES):
        hT[k, :, :NLOC] = h[k * NLOC : (k + 1) * NLOC].T.astype(ml_dtypes.bfloat16)
    iota_m = np.broadcast_to(
        np.tile(np.arange(P, dtype=np.float32), SB), (P, SB * P)
    ).astype(ml_dtypes.bfloat16)
    iota_p = np.broadcast_to(
        np.tile(np.arange(CELL, dtype=np.float32), SBPRE), (P, SBPRE * CELL)
    ).astype(ml_dtypes.bfloat16)
    b_rep = np.broadcast_to(b, (P, OUT_DIM)).copy()

    in_maps = []
    for k in range(NCORES):
        in_maps.append(
            {
                "hT_in": np.ascontiguousarray(hT[k]),
                "W_in": W.astype(ml_dtypes.bfloat16),
                "brep_in": b_rep,
                "iotam_in": np.ascontiguousarray(iota_m),
                "iotap_in": np.ascontiguousarray(iota_p),
                "idx_in": np.ascontiguousarray(idx_arr[k]),
                "mcmp_in": np.ascontiguousarray(
                    main_cmp[k].reshape(tc_main, P).T.astype(ml_dtypes.bfloat16)
                ),
                "pcmp_in": np.ascontiguousarray(
                    pre_cmp[k].reshape(tc_pre, P).T.astype(ml_dtypes.bfloat16)
                ),
            }
        )
    prof = dict(nch=nch, cmax=cmax, mch=mch, tc_main=tc_main, tc_pre=tc_pre)
    return in_maps, prof, bool(np.any(b != 0.0))


def build_program(prof, has_bias,
                  num_devices=NCORES, phases=(1, 2, 3, 4), compile=True,
                  repeat=1, ag_only=0):
    """phases: subset of {1: degree pre-pass, 2: x build, 3: allgather,
    4: main gather/scatter + epilogue}. Single-core timing variants replace
    the collective with local DMA copies."""
    nch, cmax, mch = prof["nch"], prof["cmax"], prof["mch"]
    tc_main, tc_pre = prof["tc_main"], prof["tc_pre"]
    lay = make_layout(nch, cmax)
    co, gpos, rblk, tilemax, qn, icols = (
        lay["co"], lay["gpos"], lay["rblk"], lay["tilemax"], lay["qn"],
        lay["icols"],
    )

    nc = bacc.Bacc(
        "TRN2", target_bir_lowering=False, debug=False,
        num_devices=num_devices, num_swdge_queues=NQUEUES,
    )

    hT_in = nc.dram_tensor("hT_in", [IN_DIM, NPAD], BF16, kind="ExternalInput")
    W_in = nc.dram_tensor("W_in", [IN_DIM, OUT_DIM], BF16, kind="ExternalInput")
    brep_in = nc.dram_tensor("brep_in", [P, OUT_DIM], F32, kind="ExternalInput")
    iotam_in = nc.dram_tensor("iotam_in", [P, SB * P], BF16, kind="ExternalInput")
    iotap_in = nc.dram_tensor(
        "iotap_in", [P, SBPRE * CELL], BF16, kind="ExternalInput"
    )
    idx_in = nc.dram_tensor("idx_in", [P, icols], I16, kind="ExternalInput")
    mcmp_in = nc.dram_tensor("mcmp_in", [P, tc_main], BF16, kind="ExternalInput")
    pcmp_in = nc.dram_tensor("pcmp_in", [P, tc_pre], BF16, kind="ExternalInput")
    out_dram = nc.dram_tensor("out", [NPAD, OUT_DIM], F32, kind="ExternalOutput")

    x_loc = nc.dram_tensor("x_loc", [NPAD, PITCH], BF16)
    x_full = nc.dram_tensor("x_full", [NFULL, PITCH], BF16, addr_space="Shared")

    with ExitStack() as ctx:
        tc = ctx.enter_context(tile.TileContext(nc))
        const = ctx.enter_context(tc.tile_pool(name="const", bufs=1))

        # persistent tiles
        iotam_t = const.tile([P, SB * P], BF16, tag="iotam")
        iotap_t = const.tile([P, SBPRE * CELL], BF16, tag="iotap")
        W0 = const.tile([P, OUT_DIM], BF16, tag="W0")
        W1 = const.tile([P, OUT_DIM], BF16, tag="W1")
        ones_t = const.tile([P, 1], BF16, tag="ones")
        normsrc = const.tile([P, GROUPS], F32, tag="normsrc")
        pcmp_t = const.tile([P, tc_pre], BF16, tag="pcmp")
        mcmp_t = const.tile([P, tc_main], BF16, tag="mcmp")
        idx_t = const.tile([P, icols], I16, tag="idx")
        brep_t = const.tile([P, OUT_DIM], F32, tag="brep")

        nc.sync.dma_start(out=iotam_t[:], in_=iotam_in[:, :])
        nc.sync.dma_start(out=iotap_t[:], in_=iotap_in[:, :])
        nc.sync.dma_start(out=W0[:], in_=W_in[0:P, :])
        nc.sync.dma_start(out=W1[:], in_=W_in[P : 2 * P, :])
        nc.sync.dma_start(out=pcmp_t[:], in_=pcmp_in[:, :])
        nc.sync.dma_start(out=mcmp_t[:], in_=mcmp_in[:, :])
        nc.sync.dma_start(out=idx_t[:], in_=idx_in[:, :])
        nc.sync.dma_start(out=brep_t[:], in_=brep_in[:, :])
        nc.vector.memset(ones_t[:], 1.0)

        if ag_only:
            for i in range(ag_only):
                nc.gpsimd.collective_compute(
                    "AllGather",
                    mybir.AluOpType.bypass,
                    replica_groups=[list(range(NCORES))],
                    ins=[x_loc.ap().opt()],
                    outs=[x_full.ap().opt()],
                )
                # periodically consume the AG writes so no instruction
                # (including the block-end drain) carries more than the HW
                # sync-wait limit; the x_loc write chains the next AG behind
                # the consumer (RAW), keeping wait fan-in bounded
                if (i + 1) % 6 == 0 and i < ag_only - 1:
                    nc.sync.dma_start(
                        out=x_loc.ap()[0:P, :], in_=x_full.ap()[0:P, :]
                    )
            dummy2 = const.tile([P, OUT_DIM], F32, tag="dummy2")
            nc.vector.memset(dummy2[:], 0.0)
            nc.sync.dma_start(out=out_dram[:P, :], in_=dummy2[:])
            nc.compile()
            return nc

        rep_ctx = tc.For_i(0, repeat, 1) if repeat > 1 else None
        if rep_ctx is not None:
            rep_ctx.__enter__()

        # ---------------- phase 1: out-degree pre-pass ----------------
        if 1 in phases:
          with (
            nc.named_scope("phase1_degree"),
            tc.tile_pool(name="pre_sb", bufs=4) as pre_sb,
            tc.tile_pool(name="pre_ps", bufs=8, space="PSUM") as pre_ps,
          ):
            j = 0
            Sw = None
            for g in range(GROUPS):
                deg4 = pre_sb.tile([P, 1], F32, tag="deg4")
                for sub in range(4):
                    cell = g * 4 + sub
                    nchunks = int(mch[cell])
                    dps = pre_ps.tile([CELL, 1], F32, space="PSUM", tag="dps")
                    if nchunks == 0:
                        nc.vector.memset(dps[:], 0.0)
                    for c in range(nchunks):
                        if j % SBPRE == 0:
                            w = min(SBPRE, tc_pre - j)
                            Sw = pre_sb.tile([P, SBPRE * CELL], BF16, tag="Spre")
                            nc.vector.tensor_tensor(
                                out=Sw[:, : w * CELL],
                                in0=pcmp_t[:, j : j + w].to_broadcast(
                                    [P, w, CELL]
                                ),
                                in1=iotap_t[:, : w * CELL],
                                op=mybir.AluOpType.is_equal,
                            )
                        jj = j % SBPRE
                        nc.tensor.matmul(
                            out=dps[:],
                            lhsT=Sw[:, jj * CELL : (jj + 1) * CELL],
                            rhs=ones_t[:],
                            start=(c == 0),
                            stop=(c == nchunks - 1),
                        )
                        j += 1
                    nc.vector.tensor_scalar_max(
                        deg4[CELL * sub : CELL * (sub + 1), :], dps[:], 1.0
                    )
                rcp = pre_sb.tile([P, 1], F32, tag="rcp")
                nc.vector.reciprocal(rcp[:], deg4[:])
                nc.scalar.sqrt(normsrc[:, g : g + 1], rcp[:])
        else:
            nc.vector.memset(normsrc[:], 1.0)

        # ---------------- phase 2: x = (h * norm_src) @ W, ones col ----------------
        if 2 in phases:
          with (
            nc.named_scope("phase2_xbuild"),
            tc.tile_pool(name="xb_sb", bufs=4) as xb_sb,
            tc.tile_pool(name="xb_ps", bufs=6, space="PSUM") as xb_ps,
          ):
            QUAD = 7  # groups per batched DMA (98 = 14 * 7)
            for qd in range(GROUPS // QUAD):
                g0 = qd * QUAD
                hta = xb_sb.tile([P, QUAD * P], BF16, tag="hta")
                htb = xb_sb.tile([P, QUAD * P], BF16, tag="htb")
                nc.sync.dma_start(
                    out=hta[:], in_=hT_in[0:P, g0 * P : (g0 + QUAD) * P]
                )
                nc.scalar.dma_start(
                    out=htb[:], in_=hT_in[P : 2 * P, g0 * P : (g0 + QUAD) * P]
                )
                xsb = xb_sb.tile([P, QUAD * PITCH], BF16, tag="xsb")
                nc.vector.memset(xsb[:], 1.0)
                for s in range(QUAD):
                    g = g0 + s
                    xps = xb_ps.tile([P, OUT_DIM], F32, space="PSUM", tag="xps")
                    nc.tensor.matmul(
                        out=xps[:], lhsT=hta[:, s * P : (s + 1) * P], rhs=W0[:],
                        start=True, stop=False,
                    )
                    nc.tensor.matmul(
                        out=xps[:], lhsT=htb[:, s * P : (s + 1) * P], rhs=W1[:],
                        start=False, stop=True,
                    )
                    nc.vector.tensor_scalar(
                        out=xsb[:, s * PITCH : s * PITCH + OUT_DIM],
                        in0=xps[:],
                        scalar1=normsrc[:, g : g + 1],
                        scalar2=None,
                        op0=mybir.AluOpType.mult,
                    )
                nc.sync.dma_start(
                    out=x_loc.ap()[g0 * P : (g0 + QUAD) * P, :].rearrange(
                        "(a p) d -> p a d", p=P
                    ),
                    in_=xsb[:].rearrange("p (a d) -> p a d", d=PITCH),
                )

        # ---------------- phase 3: AllGather ----------------
        if 3 in phases and repeat == 1:
          with nc.named_scope("phase3_allgather"):
            if num_devices == NCORES:
                nc.gpsimd.collective_compute(
                    "AllGather",
                    mybir.AluOpType.bypass,
                    replica_groups=[list(range(NCORES))],
                    ins=[x_loc.ap().opt()],
                    outs=[x_full.ap().opt()],
                )
            else:
                for k in range(NCORES):
                    nc.sync.dma_start(
                        out=x_full.ap()[k * NPAD : (k + 1) * NPAD, :],
                        in_=x_loc.ap()[:, :],
                    )

        # ---------------- phase 4+5: gather, scatter matmuls, epilogue ----------------
        if 4 in phases:
          with (
            nc.named_scope("phase4_main"),
            tc.tile_pool(name="mn_sb", bufs=6) as mn_sb,
            tc.tile_pool(name="mn_msg", bufs=2) as mn_msg,
            tc.tile_pool(name="mn_ps", bufs=8, space="PSUM") as mn_ps,
          ):
            OB = 7  # output groups per DMA
            ost = None
            jco = 0  # consumption-order chunk counter (S stream)
            Sw = None
            for r in range(NROUNDS):
                # one msgs tile per window for this round; one gather
                # instruction per nonempty (group, window) bucket with the
                # exact shared-profile count (skipped tail positions keep
                # stale-but-finite data; S pad columns are zero)
                mtiles = []
                for q in range(NWIN):
                    mt = mn_msg.tile([P, tilemax * D], BF16, tag=f"msg{q}")
                    mtiles.append(mt)
                    if r < 2:
                        # first-ever use of each pool buffer: clear so the
                        # never-written tail slots can't hold NaN bit patterns
                        nc.vector.memset(mt[:], 0.0)
                for g in range(r * GRANGE, (r + 1) * GRANGE):
                    for q in range(NWIN):
                        if nch[g][q] == 0:
                            continue
                        qq, col0 = qn[(g, q)]
                        blk0 = gpos[q, g] - rblk[r][q][0]
                        nb = int(nch[g][q])
                        ncols = (int(cmax[g][q]) + 15) // 16
                        raw_dma_gather(
                            nc,
                            mtiles[q][:, blk0 * D : (blk0 + nb) * D],
                            x_full[q * WIN : (q + 1) * WIN, :],
                            idx_t[:, col0 : col0 + ncols],
                            int(cmax[g][q]),
                            D,
                            PITCH * 2,
                            queue_num=qq,
                        )

                for g in range(r * GRANGE, (r + 1) * GRANGE):
                    if g % OB == 0:
                        ost = mn_sb.tile([P, OB * OUT_DIM], F32, tag="ost")
                    so = (g % OB) * OUT_DIM
                    tot = int(nch[g].sum())
                    acc = mn_ps.tile([P, D], F32, space="PSUM", tag="acc")
                    if tot == 0:
                        nc.vector.memset(acc[:], 0.0)
                    ci = 0
                    for q in range(NWIN):
                        blk0 = gpos[q, g] - rblk[r][q][0]
                        for c in range(int(nch[g][q])):
                            if jco % SB == 0:
                                Sw = mn_sb.tile([P, SB * P], BF16, tag="Smain")
                                nc.vector.tensor_tensor(
                                    out=Sw[:],
                                    in0=mcmp_t[:, jco : jco + SB].to_broadcast(
                                        [P, SB, P]
                                    ),
                                    in1=iotam_t[:],
                                    op=mybir.AluOpType.is_equal,
                                )
                            jj = jco % SB
                            bb = blk0 + c
                            nc.tensor.matmul(
                                out=acc[:],
                                lhsT=Sw[:, jj * P : (jj + 1) * P],
                                rhs=mtiles[q][:, bb * D : (bb + 1) * D],
                                start=(ci == 0),
                                stop=(ci == tot - 1),
                            )
                            jco += 1
                            ci += 1
                    deg = mn_sb.tile([P, 1], F32, tag="deg")
                    rcpm = mn_sb.tile([P, 1], F32, tag="rcpm")
                    norm = mn_sb.tile([P, 1], F32, tag="normd")
                    nc.vector.tensor_scalar_max(deg[:], acc[:, OUT_DIM:D], 1.0)
                    nc.vector.reciprocal(rcpm[:], deg[:])
                    nc.scalar.sqrt(norm[:], rcpm[:])
                    osl = ost[:, so : so + OUT_DIM]
                    if has_bias:
                        nc.vector.tensor_scalar(
                            out=osl,
                            in0=acc[:, :OUT_DIM],
                            scalar1=norm[:],
                            scalar2=None,
                            op0=mybir.AluOpType.mult,
                        )
                        nc.vector.tensor_tensor(
                            out=osl, in0=osl, in1=brep_t[:],
                            op=mybir.AluOpType.add,
                        )
                        nc.scalar.activation(
                            osl, osl, mybir.ActivationFunctionType.Relu
                        )
                    else:
                        nc.scalar.activation(
                            osl,
                            acc[:, :OUT_DIM],
                            mybir.ActivationFunctionType.Relu,
                            scale=norm[:],
                        )
                    if g % OB == OB - 1:
                        g0 = g - (OB - 1)
                        nc.sync.dma_start(
                            out=out_dram.ap()[g0 * P : (g0 + OB) * P, :].rearrange(
                                "(a p) d -> p a d", p=P
                            ),
                            in_=ost[:].rearrange("p (a d) -> p a d", d=OUT_DIM),
                        )

        if rep_ctx is not None:
            rep_ctx.__exit__(None, None, None)

    if compile:
        nc.compile()
    return nc


def kernel(h, src, dst, W, b):
    in_maps, prof, has_bias = prepare_inputs(h, src, dst, W, b)
    nc = build_program(prof, has_bias)
    res = bass_utils.run_bass_kernel_spmd(
        nc, in_maps, core_ids=list(range(NCORES))
    )
    out = np.concatenate(
        [res.results[k]["out"][:NLOC] for k in range(NCORES)], axis=0
    )
    return out.astype(np.float32)
